# revision 32
# baseline (speedup 1.0000x reference)
"""Trainium2 Bass kernel for nn_CrossAttention_68350109549162.

Math (see reference): the single K/V token makes attention softmax trivial,
so the output is

    proj = vision @ Wc + bc          (host-folded constants:
        Wc = Wv.T Wiv.T Wo.T Wout.T,  bc = bias chain, parameters only)
    out  = LayerNorm(audio + proj[:, None, :]) * gamma + beta

Sharding: pure data parallel over batch (B=32 -> 4 rows per core, 8 cores).

v6 — fp16 streaming, balanced engines, short-latency pipeline:
  * fp16 I/O (tolerance 2e-2 leaves ~20x margin): 25 MiB/core HBM traffic,
    ~70 us DMA roofline at the modeled 360 GB/s.
  * Row layout [128, 64, 768]: row r = p*64 + t, so DMAs move contiguous
    per-partition runs and the residual batch index is b = p//32.
  * Host folds the weight chain (parameters only) and replicates vision
    rows to all partitions, so the device prologue is a single fused
    matmul projB = visB @ Wc + bc — ~10 PE ops, no serial ACT ping-pong.
    wpack is split into two DMAs so the first-half matmuls start early.
  * Per sub-tile [128, 768] (DVE ~62 us, ACT ~58 us per rep, DMA ~70 us):
      DVE  tensor_scalar copy   accum -> sum(audio)   (4x fp16, pre-add so
           it only depends on the in-DMA; mean uses precomputed sum(projB))
      DVE  tensor_tensor add    x = audio + projB     (2x fp16, whole chunk)
      ACT  Square(x), accum -> sum(x^2)   [every 8th tile on DVE via
           tensor_tensor_reduce to keep ACT under the DMA bound]
      DVE  tensor_scalar        out = (x + negmean)*rstd  (4x fp16)
    Variance = E[x^2] - mean^2 (benign: mean^2 ~3e-3 vs var ~2) decouples
    the square pass from the mean.
  * TC=4 chunks (16 per rep) + two-stage skew (stats close in iter c+1,
    finals + out-DMA in iter c+2): fill/drain is ~3 chunk periods and no
    engine waits on a cross-engine result that isn't long finished.
"""

import ml_dtypes
import numpy as np

import concourse.bacc as bacc
import concourse.bass as bass
import concourse.mybir as mybir
import concourse.tile as tile
from concourse.bass_utils import run_bass_kernel_spmd

# Problem dims (hardcoded from the spec).
B, S, A, V, H = 32, 2048, 768, 512, 256
N_CORES = 8
BS = B // N_CORES          # 4 batch rows per core
P = 128                    # SBUF partitions
ROWS = BS * S              # 8192 rows per core
T = ROWS // P              # 64 row-tiles per partition
TC = 4                     # tiles per chunk (6 KiB/partition DMA runs)
NCH = T // TC              # 16 chunks per rep
KV = V // P                # 4 k-tiles over the vision dim
HALF = 384                 # matmul moving-free <= 512, so split A into 2
DVE_SQ_EVERY = 6           # every DVE_SQ_EVERY-th tile: square on DVE (TT-mult)
N_POOL_ADD = 1             # residual adds per chunk run on Pool (tile 0)
XBUFS = 10                 # x-tile ring depth
LN_EPS = 1e-5
F32 = mybir.dt.float32
F16 = mybir.dt.bfloat16

_AF = mybir.ActivationFunctionType
_OP = mybir.AluOpType

# Column offsets inside the packed fp16 constants tensor wpack [128, WF].
# Split into two DMAs: [vis | Wc half0] then [bc | Wc half1 | affine].
OFF_VIS = 0                       # visB stationary [P, KV*P] (see pack)
OFF_WC0 = OFF_VIS + KV * P        # WcT half 0 [P, KV*HALF]
CUT1 = OFF_WC0 + KV * HALF        # end of DMA 1
OFF_BC = CUT1                     # bc    [1, A] on partition 0
OFF_WC1 = OFF_BC + A              # WcT half 1 [P, KV*HALF]
WF_BASE = OFF_WC1 + KV * HALF
OFF_G = WF_BASE                   # gamma [P, A] replicated (affine only)
OFF_BETA = OFF_G + A              # beta  [P, A] replicated (affine only)


def _build(apply_affine: bool, n_reps: int = 1) -> bass.Bass:
    # n_reps > 1 repeats the main loop (same inputs/outputs) — used only by
    # test.py to measure steady-state HW time as a slope, immune to the
    # ~80 ms axon dispatch overhead. The graded path always uses n_reps=1.
    wf = WF_BASE + (2 * A if apply_affine else 0)
    nc = bacc.Bacc("TRN2", target_bir_lowering=False, debug=False, num_devices=N_CORES)

    audio = nc.dram_tensor("audio", [P, T, A], F16, kind="ExternalInput").ap()
    wpack = nc.dram_tensor("wpack", [P, wf], F16, kind="ExternalInput").ap()
    out = nc.dram_tensor("out", [P, T, A], F16, kind="ExternalOutput").ap()

    with tile.TileContext(nc) as tc:
        with (
            tc.tile_pool(name="consts", bufs=1) as consts,
            tc.tile_pool(name="pspro", bufs=2, space="PSUM") as pspro,
            tc.tile_pool(name="sqp", bufs=1, space="PSUM") as sqp,
            tc.tile_pool(name="xp", bufs=XBUFS) as xp,
            tc.tile_pool(name="stp", bufs=4) as stp,
        ):
            # ---- constants: two DMAs on the SP ring; the first audio
            # chunk's DMA is sandwiched between them (emitted in the main
            # loop below) so DVE's first row-sums start at ~6 us while the
            # second wpack half + projB matmuls finish.
            cpack = consts.tile([P, wf], F16)
            nc.sync.dma_start(out=cpack[:, :CUT1], in_=wpack[:, :CUT1])
            x0 = xp.tile([P, TC, A], F16, tag="x")
            nc.sync.dma_start(out=x0, in_=audio[:, 0:TC, :])
            nc.sync.dma_start(out=cpack[:, CUT1:], in_=wpack[:, CUT1:])

            eps_sb = consts.tile([P, 1], F32)
            nc.vector.memset(eps_sb, LN_EPS)
            onesP = consts.tile([1, P], F16)
            nc.vector.memset(onesP, 1.0)

            # ---- projB[p, :] = vision[p//32] @ Wc + bc  (one fused layer).
            # Bias matmul last so half 0 only needs the first wpack DMA.
            projB = consts.tile([P, TC, A], F16)
            for h in range(A // HALF):
                off_wc = OFF_WC0 if h == 0 else OFF_WC1
                sl = slice(h * HALF, (h + 1) * HALF)
                bp = pspro.tile([P, HALF], F32, tag=f"bc_ps{h}", bufs=1)
                for ki in range(KV):
                    nc.tensor.matmul(
                        bp,
                        cpack[:, OFF_VIS + ki * P : OFF_VIS + (ki + 1) * P],
                        cpack[:, off_wc + ki * HALF : off_wc + (ki + 1) * HALF],
                        start=(ki == 0), stop=False,
                    )
                nc.tensor.matmul(
                    bp, onesP, cpack[0:1, OFF_BC + sl.start : OFF_BC + sl.stop],
                    start=False, stop=True,
                )
                nc.scalar.copy(out=projB[:, 0, sl], in_=bp)
            for g in range(1, TC):
                nc.vector.tensor_copy(out=projB[:, g, :], in_=projB[:, 0, :])


            # ---- main loop: residual add + LayerNorm over 16 chunks ----
            import contextlib

            rep_ctx = (
                tc.For_i(
                    0, n_reps, 1,
                    hint_engines=(
                        mybir.EngineType.DVE,
                        mybir.EngineType.Activation,
                        mybir.EngineType.SP,
                        mybir.EngineType.Pool,
                    ),
                )
                if n_reps > 1
                else contextlib.nullcontext()
            )

            def tiles(st):
                """Row sums (pre-add) + residual add + square pass for chunk."""
                x = st["x"]
                ssq = st["ssq"]
                sums = st["sums"]
                c = st["c"]
                npa = N_POOL_ADD
                if npa:
                    nc.gpsimd.tensor_tensor(
                        out=x[:, :npa, :], in0=x[:, :npa, :],
                        in1=projB[:, :npa, :], op=_OP.add,
                    )
                nc.vector.tensor_tensor(
                    out=x[:, npa:, :], in0=x[:, npa:, :],
                    in1=projB[:, npa:, :], op=_OP.add,
                )
                for t in range(TC):
                    xv = x[:, t, :]
                    scr = stp.tile([P, A], F16, tag="scratch")
                    nc.vector.tensor_scalar(
                        out=scr, in0=xv, scalar1=1.0, scalar2=0.0,
                        op0=_OP.mult, op1=_OP.add,
                        accum_out=sums[:, t : t + 1],
                    )
                    if (c * TC + t) % DVE_SQ_EVERY == DVE_SQ_EVERY - 1:
                        sqd = stp.tile([P, A], F16, tag="sq_dve")
                        nc.vector.tensor_tensor(
                            out=sqd, in0=xv, in1=xv, op=_OP.mult
                        )
                        scr2 = stp.tile([P, A], F16, tag="sq_dve2")
                        nc.vector.tensor_scalar(
                            out=scr2, in0=sqd, scalar1=1.0, scalar2=0.0,
                            op0=_OP.mult, op1=_OP.add,
                            accum_out=ssq[:, t : t + 1],
                        )
                    else:
                        sq = sqp.tile([P, A], F32, tag="sq")
                        nc.scalar.activation(
                            out=sq, in_=xv, func=_AF.Square,
                            bias=0.0, scale=1.0,
                            accum_out=ssq[:, t : t + 1],
                        )

            def stats_a(st):
                """negmean / var_raw on DVE (chunk c-1)."""
                negmean = stp.tile([P, TC], F32, tag="negmean")
                nc.vector.tensor_scalar(
                    out=negmean, in0=st["sums"], scalar1=-1.0 / A, scalar2=None,
                    op0=_OP.mult,
                )
                msq = stp.tile([P, TC], F32, tag="msq")
                nc.vector.tensor_tensor(
                    out=msq, in0=negmean, in1=negmean, op=_OP.mult
                )
                var_raw = stp.tile([P, TC], F32, tag="var_raw")
                nc.vector.scalar_tensor_tensor(
                    out=var_raw, in0=msq, scalar=-float(A), in1=st["ssq"],
                    op0=_OP.mult, op1=_OP.add,
                )
                st["negmean"] = negmean
                st["var_raw"] = var_raw

            def stats_sd(st):
                """sd = sqrt(var_raw/A + eps) on ACT (chunk c-2)."""
                sd = stp.tile([P, TC], F32, tag="sd")
                nc.scalar.activation(
                    out=sd, in_=st["var_raw"], func=_AF.Sqrt,
                    bias=eps_sb, scale=1.0 / A,
                )
                st["sd"] = sd

            def finals(st):
                """rstd + normalize + out-DMA (chunk c-2)."""
                rstd = stp.tile([P, TC], F32, tag="rstd")
                nc.vector.reciprocal(out=rstd, in_=st["sd"])
                x = st["x"]
                negmean = st["negmean"]
                for t in range(TC):
                    xv = x[:, t, :]
                    nc.vector.tensor_scalar(
                        out=xv, in0=xv,
                        scalar1=negmean[:, t : t + 1], scalar2=rstd[:, t : t + 1],
                        op0=_OP.add, op1=_OP.mult,
                    )
                    if apply_affine:
                        nc.vector.tensor_tensor(
                            out=xv, in0=xv, in1=cpack[:, OFF_G : OFF_G + A],
                            op=_OP.mult,
                        )
                        nc.vector.tensor_tensor(
                            out=xv, in0=xv, in1=cpack[:, OFF_BETA : OFF_BETA + A],
                            op=_OP.add,
                        )
                c = st["c"]
                hc = TC // 2
                if c >= NCH - 2:
                    # split the tail chunks' outs so the last transfer is
                    # small — it sits on the critical path after the last
                    # finals.
                    for hh in range(2):
                        nc.gpsimd.dma_start(
                            out=out[:, c * TC + hh * hc : c * TC + (hh + 1) * hc, :],
                            in_=x[:, hh * hc : (hh + 1) * hc, :],
                        )
                else:
                    nc.gpsimd.dma_start(
                        out=out[:, c * TC : (c + 1) * TC, :], in_=x
                    )

            with rep_ctx:
                hist = []
                for c in range(NCH):
                    if c == 0:
                        x = x0
                        if n_reps > 1:
                            # reload chunk 0 every rep (the prologue load
                            # only covers the first iteration)
                            nc.sync.dma_start(out=x0, in_=audio[:, 0:TC, :])
                    else:
                        x = xp.tile([P, TC, A], F16, tag="x")
                        nc.sync.dma_start(
                            out=x, in_=audio[:, c * TC : (c + 1) * TC, :]
                        )
                    sums = stp.tile([P, TC], F32, tag="sums")
                    ssq = stp.tile([P, TC], F32, tag="ssq")
                    st = {"c": c, "x": x, "sums": sums, "ssq": ssq}
                    if len(hist) >= 2:
                        stats_sd(hist[-2])
                    tiles(st)
                    if len(hist) >= 1:
                        stats_a(hist[-1])
                    if len(hist) >= 2:
                        finals(hist[-2])
                    hist.append(st)
                # drain the 2-deep pipeline
                stats_a(hist[-1])
                stats_sd(hist[-2])
                finals(hist[-2])
                stats_sd(hist[-1])
                finals(hist[-1])

    nc.compile()
    return nc


_nc_cache: dict = {}


def _get_nc(apply_affine: bool, n_reps: int = 1) -> bass.Bass:
    key = (apply_affine, n_reps)
    if key not in _nc_cache:
        _nc_cache[key] = _build(apply_affine, n_reps)
    return _nc_cache[key]


def make_in_maps(inputs: dict) -> tuple[list, bool]:
    """Host-side prep: slice batch per core, fold weights, pack fp16."""
    f = lambda k: np.asarray(inputs[k], dtype=np.float64)
    audio = np.asarray(inputs["audio_features"])
    vision = f("vision_features")
    gamma = np.asarray(inputs["gamma"], np.float32)
    beta = np.asarray(inputs["beta"], np.float32)
    apply_affine = not (np.all(gamma == 1.0) and np.all(beta == 0.0))
    wf = WF_BASE + (2 * A if apply_affine else 0)

    # Fold the constant weight chain (parameters only, no input data):
    #   Wc = Wv.T @ Wiv.T @ Wo.T @ Wout.T,  bc = bias chain through the same.
    Wiv = f("in_proj_w")[2 * H :]
    biv = f("in_proj_b")[2 * H :]
    Wc = f("Wv").T @ Wiv.T @ f("Wo_mha").T @ f("Wout").T          # [V, A]
    bc = (
        ((f("bv") @ Wiv.T + biv) @ f("Wo_mha").T + f("bo_mha")) @ f("Wout").T
        + f("bout")
    )                                                             # [A]

    base = np.zeros((P, wf), ml_dtypes.bfloat16)
    # WcT half-major: base[p, OFF_WCh + ki*HALF + j] = Wc[ki*P + p, h*HALF+j]
    Wk = Wc.reshape(KV, P, A).transpose(1, 0, 2)                  # [P, KV, A]
    base[:, OFF_WC0 : OFF_WC0 + KV * HALF] = Wk[:, :, :HALF].reshape(
        P, KV * HALF
    )
    base[:, OFF_WC1 : OFF_WC1 + KV * HALF] = Wk[:, :, HALF:].reshape(
        P, KV * HALF
    )
    base[0, OFF_BC : OFF_BC + A] = bc
    if apply_affine:
        base[:, OFF_G : OFF_G + A] = gamma[None, :]
        base[:, OFF_BETA : OFF_BETA + A] = beta[None, :]

    in_maps = []
    for c in range(N_CORES):
        sl = slice(c * BS, (c + 1) * BS)
        wpack = base.copy()
        # visB stationary (lhsT): wpack[k, ki*P + m] = vision[m//32, ki*P + k]
        visTrep = vision[sl].T[:, np.arange(P) // (P // BS)]      # [V, P]
        wpack[:, OFF_VIS : OFF_VIS + KV * P] = (
            visTrep.reshape(KV, P, P).transpose(1, 0, 2).reshape(P, KV * P)
        )
        in_maps.append({
            "audio": np.ascontiguousarray(audio[sl], dtype=ml_dtypes.bfloat16).reshape(
                P, T, A
            ),
            "wpack": wpack,
        })
    return in_maps, apply_affine


def kernel(**inputs) -> np.ndarray:
    in_maps, apply_affine = make_in_maps(inputs)
    nc = _get_nc(apply_affine)
    res = run_bass_kernel_spmd(nc, in_maps, core_ids=list(range(N_CORES)))
    return np.concatenate(
        [r["out"].reshape(BS, S, A) for r in res.results], axis=0
    ).astype(np.float32)


# revision 33
# speedup vs baseline: 2.2552x; 2.2552x over previous
"""Trainium2 Bass kernel for nn_CrossAttention_68350109549162.

Math (see reference): the single K/V token makes attention softmax trivial,
so the output is

    proj = vision @ Wc + bc          (host-folded constants:
        Wc = Wv.T Wiv.T Wo.T Wout.T,  bc = bias chain, parameters only)
    out  = LayerNorm(audio + proj[:, None, :]) * gamma + beta

Sharding: pure data parallel over batch (B=32 -> 4 rows per core, 8 cores).

v10 — bf16 streaming, four engines balanced, short-latency pipeline:
  * bf16 I/O (tolerance 2e-2; measured rel err 6e-3 on the fixed seed):
    25 MiB/core HBM traffic, ~70 us DMA roofline at the modeled 360 GB/s.
    bf16 (not fp16) because the GPSIMD Q7 engine only reads fp32/bf16,
    which lets Pool take one residual add per chunk.
  * Row layout [128, 64, 768]: row r = p*64 + t, so DMAs move contiguous
    per-partition runs and the residual batch index is b = p//32.
  * Host folds the weight chain (parameters only) and replicates vision
    rows to all partitions, so the device prologue is a single fused
    matmul projB = visB @ Wc + bc — ~10 PE ops, no serial ACT ping-pong.
    wpack is split into two DMAs so the first-half matmuls start early.
  * Per sub-tile [128, 768] (sim: DVE ~66 us, ACT ~61 us, Pool ~45 us,
    DMA ~73 us per rep):
      DVE/Pool tensor_tensor add  x = audio + projB  (2x bf16; tile 0 of
               each chunk on Pool)
      DVE  tensor_scalar copy     accum -> sum(x)    (4x bf16)
      ACT  Square(x), accum -> sum(x^2)  [every 6th tile on DVE as
           TT-mult + ts-accum; NOTE tensor_tensor_reduce compiles and
           passes CoreSim but CRASHES on HW — do not use]
      DVE  tensor_scalar          out = (x + negmean)*rstd  (4x bf16)
    Variance = E[x^2] - mean^2 (benign: mean^2 ~3e-3 vs var ~2) decouples
    the square pass from the mean.
  * TC=4 chunks (16 per rep) + two-stage skew (stats close in iter c+1,
    finals + out-DMA in iter c+2): fill/drain is ~3 chunk periods and no
    engine waits on a cross-engine result that isn't long finished.
"""

import ml_dtypes
import numpy as np

import concourse.bacc as bacc
import concourse.bass as bass
import concourse.mybir as mybir
import concourse.tile as tile
from concourse.bass_utils import run_bass_kernel_spmd

# Problem dims (hardcoded from the spec).
B, S, A, V, H = 32, 2048, 768, 512, 256
N_CORES = 8
BS = B // N_CORES          # 4 batch rows per core
P = 128                    # SBUF partitions
ROWS = BS * S              # 8192 rows per core
T = ROWS // P              # 64 row-tiles per partition
TC = 4                     # tiles per chunk (6 KiB/partition DMA runs)
NCH = T // TC              # 16 chunks per rep
KV = V // P                # 4 k-tiles over the vision dim
HALF = 384                 # matmul moving-free <= 512, so split A into 2
DVE_SQ_EVERY = 6           # every DVE_SQ_EVERY-th tile: square on DVE (TT-mult)
N_POOL_ADD = 1             # residual adds per chunk run on Pool (tile 0)
XBUFS = 10                 # x-tile ring depth
LN_EPS = 1e-5
F32 = mybir.dt.float32
F16 = mybir.dt.bfloat16

_AF = mybir.ActivationFunctionType
_OP = mybir.AluOpType

# Column offsets inside the packed fp16 constants tensor wpack [128, WF].
# Split into two DMAs: [vis | Wc half0] then [bc | Wc half1 | affine].
OFF_VIS = 0                       # visB stationary [P, KV*P] (see pack)
OFF_WC0 = OFF_VIS + KV * P        # WcT half 0 [P, KV*HALF]
CUT1 = OFF_WC0 + KV * HALF        # end of DMA 1
OFF_BC = CUT1                     # bc    [1, A] on partition 0
OFF_WC1 = OFF_BC + A              # WcT half 1 [P, KV*HALF]
WF_BASE = OFF_WC1 + KV * HALF
OFF_G = WF_BASE                   # gamma [P, A] replicated (affine only)
OFF_BETA = OFF_G + A              # beta  [P, A] replicated (affine only)


def _build(apply_affine: bool, n_reps: int = 1) -> bass.Bass:
    # n_reps > 1 repeats the main loop (same inputs/outputs) — used only by
    # test.py to measure steady-state HW time as a slope, immune to the
    # ~80 ms axon dispatch overhead. The graded path always uses n_reps=1.
    wf = WF_BASE + (2 * A if apply_affine else 0)
    nc = bacc.Bacc("TRN2", target_bir_lowering=False, debug=False, num_devices=N_CORES)

    audio = nc.dram_tensor("audio", [P, T, A], F16, kind="ExternalInput").ap()
    wpack = nc.dram_tensor("wpack", [P, wf], F16, kind="ExternalInput").ap()
    out = nc.dram_tensor("out", [P, T, A], F16, kind="ExternalOutput").ap()

    with tile.TileContext(nc) as tc:
        with (
            tc.tile_pool(name="consts", bufs=1) as consts,
            tc.tile_pool(name="pspro", bufs=2, space="PSUM") as pspro,
            tc.tile_pool(name="sqp", bufs=1, space="PSUM") as sqp,
            tc.tile_pool(name="xp", bufs=XBUFS) as xp,
            tc.tile_pool(name="stp", bufs=4) as stp,
        ):
            # ---- constants: two DMAs on the SP ring; the first audio
            # chunk's DMA is sandwiched between them (emitted in the main
            # loop below) so DVE's first row-sums start at ~6 us while the
            # second wpack half + projB matmuls finish.
            cpack = consts.tile([P, wf], F16)
            nc.sync.dma_start(out=cpack[:, :CUT1], in_=wpack[:, :CUT1])
            x0 = xp.tile([P, TC, A], F16, tag="x")
            nc.sync.dma_start(out=x0, in_=audio[:, 0:TC, :])
            nc.sync.dma_start(out=cpack[:, CUT1:], in_=wpack[:, CUT1:])

            eps_sb = consts.tile([P, 1], F32)
            nc.vector.memset(eps_sb, LN_EPS)
            onesP = consts.tile([1, P], F16)
            nc.vector.memset(onesP, 1.0)

            # ---- projB[p, :] = vision[p//32] @ Wc + bc  (one fused layer).
            # Bias matmul last so half 0 only needs the first wpack DMA.
            projB = consts.tile([P, TC, A], F16)
            for h in range(A // HALF):
                off_wc = OFF_WC0 if h == 0 else OFF_WC1
                sl = slice(h * HALF, (h + 1) * HALF)
                bp = pspro.tile([P, HALF], F32, tag=f"bc_ps{h}", bufs=1)
                for ki in range(KV):
                    nc.tensor.matmul(
                        bp,
                        cpack[:, OFF_VIS + ki * P : OFF_VIS + (ki + 1) * P],
                        cpack[:, off_wc + ki * HALF : off_wc + (ki + 1) * HALF],
                        start=(ki == 0), stop=False,
                    )
                nc.tensor.matmul(
                    bp, onesP, cpack[0:1, OFF_BC + sl.start : OFF_BC + sl.stop],
                    start=False, stop=True,
                )
                nc.scalar.copy(out=projB[:, 0, sl], in_=bp)
            for g in range(1, TC):
                nc.vector.tensor_copy(out=projB[:, g, :], in_=projB[:, 0, :])


            # ---- main loop: residual add + LayerNorm over 16 chunks ----
            import contextlib

            rep_ctx = (
                tc.For_i(
                    0, n_reps, 1,
                    hint_engines=(
                        mybir.EngineType.DVE,
                        mybir.EngineType.Activation,
                        mybir.EngineType.SP,
                        mybir.EngineType.Pool,
                    ),
                )
                if n_reps > 1
                else contextlib.nullcontext()
            )

            def tiles(st):
                """Row sums (pre-add) + residual add + square pass for chunk."""
                x = st["x"]
                ssq = st["ssq"]
                sums = st["sums"]
                c = st["c"]
                npa = N_POOL_ADD
                if npa:
                    nc.gpsimd.tensor_tensor(
                        out=x[:, :npa, :], in0=x[:, :npa, :],
                        in1=projB[:, :npa, :], op=_OP.add,
                    )
                nc.vector.tensor_tensor(
                    out=x[:, npa:, :], in0=x[:, npa:, :],
                    in1=projB[:, npa:, :], op=_OP.add,
                )
                for t in range(TC):
                    xv = x[:, t, :]
                    scr = stp.tile([P, A], F16, tag="scratch")
                    nc.vector.tensor_scalar(
                        out=scr, in0=xv, scalar1=1.0, scalar2=0.0,
                        op0=_OP.mult, op1=_OP.add,
                        accum_out=sums[:, t : t + 1],
                    )
                    if (c * TC + t) % DVE_SQ_EVERY == DVE_SQ_EVERY - 1:
                        sqd = stp.tile([P, A], F16, tag="sq_dve")
                        nc.vector.tensor_tensor(
                            out=sqd, in0=xv, in1=xv, op=_OP.mult
                        )
                        scr2 = stp.tile([P, A], F16, tag="sq_dve2")
                        nc.vector.tensor_scalar(
                            out=scr2, in0=sqd, scalar1=1.0, scalar2=0.0,
                            op0=_OP.mult, op1=_OP.add,
                            accum_out=ssq[:, t : t + 1],
                        )
                    else:
                        sq = sqp.tile([P, A], F32, tag="sq")
                        nc.scalar.activation(
                            out=sq, in_=xv, func=_AF.Square,
                            bias=0.0, scale=1.0,
                            accum_out=ssq[:, t : t + 1],
                        )

            def stats_a(st):
                """negmean / var_raw on DVE (chunk c-1)."""
                negmean = stp.tile([P, TC], F32, tag="negmean")
                nc.vector.tensor_scalar(
                    out=negmean, in0=st["sums"], scalar1=-1.0 / A, scalar2=None,
                    op0=_OP.mult,
                )
                msq = stp.tile([P, TC], F32, tag="msq")
                nc.vector.tensor_tensor(
                    out=msq, in0=negmean, in1=negmean, op=_OP.mult
                )
                var_raw = stp.tile([P, TC], F32, tag="var_raw")
                nc.vector.scalar_tensor_tensor(
                    out=var_raw, in0=msq, scalar=-float(A), in1=st["ssq"],
                    op0=_OP.mult, op1=_OP.add,
                )
                st["negmean"] = negmean
                st["var_raw"] = var_raw

            def stats_sd(st):
                """sd = sqrt(var_raw/A + eps) on ACT (chunk c-2)."""
                sd = stp.tile([P, TC], F32, tag="sd")
                nc.scalar.activation(
                    out=sd, in_=st["var_raw"], func=_AF.Sqrt,
                    bias=eps_sb, scale=1.0 / A,
                )
                st["sd"] = sd

            def finals(st):
                """rstd + normalize + out-DMA (chunk c-2)."""
                rstd = stp.tile([P, TC], F32, tag="rstd")
                nc.vector.reciprocal(out=rstd, in_=st["sd"])
                x = st["x"]
                negmean = st["negmean"]
                for t in range(TC):
                    xv = x[:, t, :]
                    nc.vector.tensor_scalar(
                        out=xv, in0=xv,
                        scalar1=negmean[:, t : t + 1], scalar2=rstd[:, t : t + 1],
                        op0=_OP.add, op1=_OP.mult,
                    )
                    if apply_affine:
                        nc.vector.tensor_tensor(
                            out=xv, in0=xv, in1=cpack[:, OFF_G : OFF_G + A],
                            op=_OP.mult,
                        )
                        nc.vector.tensor_tensor(
                            out=xv, in0=xv, in1=cpack[:, OFF_BETA : OFF_BETA + A],
                            op=_OP.add,
                        )
                c = st["c"]
                hc = TC // 2
                if c >= NCH - 2:
                    # split the tail chunks' outs so the last transfer is
                    # small — it sits on the critical path after the last
                    # finals.
                    for hh in range(2):
                        nc.gpsimd.dma_start(
                            out=out[:, c * TC + hh * hc : c * TC + (hh + 1) * hc, :],
                            in_=x[:, hh * hc : (hh + 1) * hc, :],
                        )
                else:
                    nc.gpsimd.dma_start(
                        out=out[:, c * TC : (c + 1) * TC, :], in_=x
                    )

            with rep_ctx:
                hist = []
                for c in range(NCH):
                    if c == 0:
                        x = x0
                        if n_reps > 1:
                            # reload chunk 0 every rep (the prologue load
                            # only covers the first iteration)
                            nc.sync.dma_start(out=x0, in_=audio[:, 0:TC, :])
                    else:
                        x = xp.tile([P, TC, A], F16, tag="x")
                        nc.sync.dma_start(
                            out=x, in_=audio[:, c * TC : (c + 1) * TC, :]
                        )
                    sums = stp.tile([P, TC], F32, tag="sums")
                    ssq = stp.tile([P, TC], F32, tag="ssq")
                    st = {"c": c, "x": x, "sums": sums, "ssq": ssq}
                    if len(hist) >= 2:
                        stats_sd(hist[-2])
                    tiles(st)
                    if len(hist) >= 1:
                        stats_a(hist[-1])
                    if len(hist) >= 2:
                        finals(hist[-2])
                    hist.append(st)
                # drain the 2-deep pipeline
                stats_a(hist[-1])
                stats_sd(hist[-2])
                finals(hist[-2])
                stats_sd(hist[-1])
                finals(hist[-1])

    nc.compile()
    return nc


_nc_cache: dict = {}


def _get_nc(apply_affine: bool, n_reps: int = 1) -> bass.Bass:
    key = (apply_affine, n_reps)
    if key not in _nc_cache:
        _nc_cache[key] = _build(apply_affine, n_reps)
    return _nc_cache[key]


def make_in_maps(inputs: dict) -> tuple[list, bool]:
    """Host-side prep: slice batch per core, fold weights, pack fp16."""
    f = lambda k: np.asarray(inputs[k], dtype=np.float64)
    audio = np.asarray(inputs["audio_features"])
    vision = f("vision_features")
    gamma = np.asarray(inputs["gamma"], np.float32)
    beta = np.asarray(inputs["beta"], np.float32)
    apply_affine = not (np.all(gamma == 1.0) and np.all(beta == 0.0))
    wf = WF_BASE + (2 * A if apply_affine else 0)

    # Fold the constant weight chain (parameters only, no input data):
    #   Wc = Wv.T @ Wiv.T @ Wo.T @ Wout.T,  bc = bias chain through the same.
    Wiv = f("in_proj_w")[2 * H :]
    biv = f("in_proj_b")[2 * H :]
    Wc = f("Wv").T @ Wiv.T @ f("Wo_mha").T @ f("Wout").T          # [V, A]
    bc = (
        ((f("bv") @ Wiv.T + biv) @ f("Wo_mha").T + f("bo_mha")) @ f("Wout").T
        + f("bout")
    )                                                             # [A]

    base = np.zeros((P, wf), ml_dtypes.bfloat16)
    # WcT half-major: base[p, OFF_WCh + ki*HALF + j] = Wc[ki*P + p, h*HALF+j]
    Wk = Wc.reshape(KV, P, A).transpose(1, 0, 2)                  # [P, KV, A]
    base[:, OFF_WC0 : OFF_WC0 + KV * HALF] = Wk[:, :, :HALF].reshape(
        P, KV * HALF
    )
    base[:, OFF_WC1 : OFF_WC1 + KV * HALF] = Wk[:, :, HALF:].reshape(
        P, KV * HALF
    )
    base[0, OFF_BC : OFF_BC + A] = bc
    if apply_affine:
        base[:, OFF_G : OFF_G + A] = gamma[None, :]
        base[:, OFF_BETA : OFF_BETA + A] = beta[None, :]

    in_maps = []
    for c in range(N_CORES):
        sl = slice(c * BS, (c + 1) * BS)
        wpack = base.copy()
        # visB stationary (lhsT): wpack[k, ki*P + m] = vision[m//32, ki*P + k]
        visTrep = vision[sl].T[:, np.arange(P) // (P // BS)]      # [V, P]
        wpack[:, OFF_VIS : OFF_VIS + KV * P] = (
            visTrep.reshape(KV, P, P).transpose(1, 0, 2).reshape(P, KV * P)
        )
        in_maps.append({
            "audio": np.ascontiguousarray(audio[sl], dtype=ml_dtypes.bfloat16).reshape(
                P, T, A
            ),
            "wpack": wpack,
        })
    return in_maps, apply_affine


def kernel(**inputs) -> np.ndarray:
    in_maps, apply_affine = make_in_maps(inputs)
    nc = _get_nc(apply_affine)
    res = run_bass_kernel_spmd(nc, in_maps, core_ids=list(range(N_CORES)))
    return np.concatenate(
        [r["out"].reshape(BS, S, A) for r in res.results], axis=0
    ).astype(np.float32)
